# revision 6
# baseline (speedup 1.0000x reference)
"""Trainium2 Bass kernel for nn_Disease_Guide_ROI (dense_transformer).

Math notes (verified vs reference numerically):
  - softmax over a length-1 axis is exactly 1.0 => attention collapses to
    x1 = v * weight; q/k/cls_out/cls_w/cls_b are dead.
  - only the v half of the kv projection is needed.
  - the GRU update after iteration 3 is dead (weight3 unused).
  - iteration-1 gates are affine in x (hidden = w0 const): host-compose
    W_g1 = w_ih_g @ diag(w0) @ kv_v so they come straight from x.
  - with zc = 1-z (sigmoid at scale=-1):  w1 = w0 + zc1*(n1-w0),
    w2 = w1 + zc2*(n2-w1).

Precision: fp16 working tiles (~8e-4 end-to-end rel err, validated in a host
simulation), fp32 PSUM accumulation, fp32 final output.
PE observation: this environment pins the PE at 1.2 GHz and every dtype
streams 1 col/cycle, so performance = matmul count; fp16 halves DVE cost
(2x/4x modes) and input DMA bytes.
Layout: channel-major [90, N]; host pre-transposes x to (90, B) and
post-transposes the (90, B) output so every DMA moves contiguous rows.
Sharding: pure data parallel, B/8 = 16384 samples per core.
"""

import sys

if "/opt/trn_rl_repo" not in sys.path:
    sys.path.insert(0, "/opt/trn_rl_repo")

import numpy as np
from contextlib import ExitStack

B = 131072
C = 90
NCORES = 8
BC = B // NCORES  # 16384
CHUNK = 1024
NCHUNK = BC // CHUNK  # 16
MMN = 512  # matmul moving free dim (one fp32 PSUM bank)

# column indices into the per-partition fp32 constant tensor
(CV_BV, CV_W0, CV_HN1, CV_BR1, CV_NBZ1, CV_BN1, CV_BR2, CV_NBZ2, CV_BIHN,
 CV_BHHN, CV_BP) = range(11)
NCV = 11
NW = 11  # [90,90] lhsT matrices stacked in wmat

_BUILD_CACHE = {}


def _build_nc():
    import concourse.bacc as bacc
    import concourse.tile as tile
    import concourse.mybir as mybir

    f32 = mybir.dt.float32
    f16 = mybir.dt.float16
    Alu = mybir.AluOpType
    Act = mybir.ActivationFunctionType

    nc = bacc.Bacc(None, target_bir_lowering=False)
    with ExitStack() as ctx:
        tc = ctx.enter_context(tile.TileContext(nc))
        xT = nc.dram_tensor("xT", [C, BC], f16, kind="ExternalInput")
        wmat = nc.dram_tensor("wmat", [C, NW * C], f16, kind="ExternalInput")
        cvec = nc.dram_tensor("cvec", [C, NCV], f32, kind="ExternalInput")
        outT = nc.dram_tensor("outT", [C, BC], f32, kind="ExternalOutput")

        const = ctx.enter_context(tc.tile_pool(name="const", bufs=1))
        io = ctx.enter_context(tc.tile_pool(name="io", bufs=3))
        work = ctx.enter_context(tc.tile_pool(name="work", bufs=4))
        ps = ctx.enter_context(tc.tile_pool(name="ps", bufs=3, space="PSUM"))

        Wm = const.tile([C, NW * C], f16)
        nc.sync.dma_start(out=Wm, in_=wmat[:, :])
        cv = const.tile([C, NCV], f32)
        nc.sync.dma_start(out=cv, in_=cvec[:, :])

        (kvT, W1rT, W1zT, W1nT, wihT_r, wihT_z, wihT_n,
         whhT_r, whhT_z, whhT_n, projT) = (
            Wm[:, i * C:(i + 1) * C] for i in range(NW))

        def col(i):
            return cv[:, i:i + 1]

        def mm2(out_t, lhsT, rhs, start=True, stop=True):
            for h in range(CHUNK // MMN):
                nc.tensor.matmul(out_t[:, h * MMN:(h + 1) * MMN], lhsT,
                                 rhs[:, h * MMN:(h + 1) * MMN],
                                 start=start, stop=stop)

        for ch in range(NCHUNK):
            sl = slice(ch * CHUNK, (ch + 1) * CHUNK)

            x_h = io.tile([C, CHUNK], f16, tag="x")
            nc.sync.dma_start(out=x_h, in_=xT[:, sl])

            # ---- v projection ----
            pv = ps.tile([C, CHUNK], f32, tag="g")
            mm2(pv, kvT, x_h)
            v = work.tile([C, CHUNK], f16, tag="v")
            nc.scalar.activation(v, pv, Act.Identity, bias=col(CV_BV))

            # ---- iteration 1 (gates straight from x; hidden = w0) ----
            pr1 = ps.tile([C, CHUNK], f32, tag="g")
            mm2(pr1, W1rT, x_h)
            r1 = work.tile([C, CHUNK], f16, tag="g1")
            nc.scalar.activation(r1, pr1, Act.Sigmoid, bias=col(CV_BR1))

            pz1 = ps.tile([C, CHUNK], f32, tag="g")
            mm2(pz1, W1zT, x_h)
            zc1 = work.tile([C, CHUNK], f16, tag="g1")
            nc.scalar.activation(zc1, pz1, Act.Sigmoid, bias=col(CV_NBZ1),
                                 scale=-1.0)

            pi1 = ps.tile([C, CHUNK], f32, tag="g")
            mm2(pi1, W1nT, x_h)
            mt = work.tile([C, CHUNK], f16, tag="g1")
            nc.gpsimd.tensor_scalar(mt, r1, col(CV_HN1), None, Alu.mult)
            t2 = work.tile([C, CHUNK], f16, tag="g1")
            nc.vector.tensor_tensor(t2, mt, pi1, Alu.add)
            n1 = work.tile([C, CHUNK], f16, tag="g1")
            nc.scalar.activation(n1, t2, Act.Tanh, bias=col(CV_BN1))

            # m' = zc1*(n1 - w0);  w1 = w0 + m'
            mp = work.tile([C, CHUNK], f16, tag="g1")
            nc.vector.scalar_tensor_tensor(
                mp, n1, col(CV_W0), zc1, Alu.subtract, Alu.mult)
            w1 = work.tile([C, CHUNK], f16, tag="w")
            nc.vector.tensor_scalar(w1, mp, col(CV_W0), None, Alu.add)
            x1b = work.tile([C, CHUNK], f16, tag="x1")
            nc.vector.tensor_tensor(x1b, v, w1, Alu.mult)

            # ---- iteration 2 ----
            pr2 = ps.tile([C, CHUNK], f32, tag="g")
            mm2(pr2, wihT_r, x1b, start=True, stop=False)
            mm2(pr2, whhT_r, w1, start=False, stop=True)
            r2 = work.tile([C, CHUNK], f16, tag="g2")
            nc.scalar.activation(r2, pr2, Act.Sigmoid, bias=col(CV_BR2))

            pz2 = ps.tile([C, CHUNK], f32, tag="g")
            mm2(pz2, wihT_z, x1b, start=True, stop=False)
            mm2(pz2, whhT_z, w1, start=False, stop=True)
            zc2 = work.tile([C, CHUNK], f16, tag="g2")
            nc.scalar.activation(zc2, pz2, Act.Sigmoid, bias=col(CV_NBZ2),
                                 scale=-1.0)

            pi2 = ps.tile([C, CHUNK], f32, tag="g")
            mm2(pi2, wihT_n, x1b)
            ph2 = ps.tile([C, CHUNK], f32, tag="g")
            mm2(ph2, whhT_n, w1)

            t = work.tile([C, CHUNK], f16, tag="g2")
            nc.vector.scalar_tensor_tensor(
                t, ph2, col(CV_BHHN), r2, Alu.add, Alu.mult)
            t2b = work.tile([C, CHUNK], f16, tag="g2")
            nc.vector.tensor_tensor(t2b, t, pi2, Alu.add)
            n2 = work.tile([C, CHUNK], f16, tag="g2")
            nc.scalar.activation(n2, t2b, Act.Tanh, bias=col(CV_BIHN))

            # w2 = w1 + zc2*(n2 - w1);  x1c = v*w2
            u2 = work.tile([C, CHUNK], f16, tag="g2")
            nc.gpsimd.tensor_tensor(u2, n2, w1, Alu.subtract)
            m2 = work.tile([C, CHUNK], f16, tag="g2")
            nc.gpsimd.tensor_tensor(m2, zc2, u2, Alu.mult)
            w2 = work.tile([C, CHUNK], f16, tag="w")
            nc.vector.tensor_tensor(w2, w1, m2, Alu.add)
            x1c = work.tile([C, CHUNK], f16, tag="x1")
            nc.vector.tensor_tensor(x1c, v, w2, Alu.mult)

            # ---- output projection ----
            po = ps.tile([C, CHUNK], f32, tag="po", bufs=1)
            mm2(po, projT, x1c)
            o = io.tile([C, CHUNK], f32, tag="o")
            nc.vector.tensor_scalar(o, po, col(CV_BP), None, Alu.add)
            nc.sync.dma_start(out=outT[:, sl], in_=o)

    nc.compile()
    return nc


def _get_nc():
    if "nc" not in _BUILD_CACHE:
        _BUILD_CACHE["nc"] = _build_nc()
    return _BUILD_CACHE["nc"]


def _prep_consts(w0, kv_w, kv_b, w_ih, w_hh, b_ih, b_hh, proj_w, proj_b):
    f8 = np.float64
    w0v = np.asarray(w0, f8).reshape(C)
    kv_w = np.asarray(kv_w, f8)
    kv_b = np.asarray(kv_b, f8)
    w_ih = np.asarray(w_ih, f8)
    w_hh = np.asarray(w_hh, f8)
    b_ih = np.asarray(b_ih, f8)
    b_hh = np.asarray(b_hh, f8)
    proj_w = np.asarray(proj_w, f8)
    proj_b = np.asarray(proj_b, f8)

    kv_v = kv_w[C:2 * C]
    b_v = kv_b[C:2 * C]
    gh1 = w0v @ w_hh.T + b_hh  # iter-1 hidden gate contribution (const)

    wg = {}
    for i, g in enumerate(("r", "z", "n")):
        wg[g] = (w_ih[i * C:(i + 1) * C] * w0v[None, :]) @ kv_v

    mats = [
        kv_v.T,
        wg["r"].T, wg["z"].T, wg["n"].T,
        w_ih[0:C].T, w_ih[C:2 * C].T, w_ih[2 * C:3 * C].T,
        w_hh[0:C].T, w_hh[C:2 * C].T, w_hh[2 * C:3 * C].T,
        proj_w.T,
    ]
    wmat = np.ascontiguousarray(
        np.concatenate(mats, axis=1).astype(np.float16))

    bgate1 = {g: w_ih[i * C:(i + 1) * C] @ (w0v * b_v) + b_ih[i * C:(i + 1) * C]
              for i, g in enumerate(("r", "z", "n"))}
    cvec = np.zeros((C, NCV), np.float32)
    cvec[:, CV_BV] = b_v
    cvec[:, CV_W0] = w0v
    cvec[:, CV_HN1] = gh1[2 * C:3 * C]
    cvec[:, CV_BR1] = bgate1["r"] + gh1[0:C]
    cvec[:, CV_NBZ1] = -(bgate1["z"] + gh1[C:2 * C])
    cvec[:, CV_BN1] = bgate1["n"]
    cvec[:, CV_BR2] = b_ih[0:C] + b_hh[0:C]
    cvec[:, CV_NBZ2] = -(b_ih[C:2 * C] + b_hh[C:2 * C])
    cvec[:, CV_BIHN] = b_ih[2 * C:3 * C]
    cvec[:, CV_BHHN] = b_hh[2 * C:3 * C]
    cvec[:, CV_BP] = proj_b
    return wmat, cvec


def _run(inputs, trace=False):
    from concourse.bass_utils import run_bass_kernel_spmd

    x = np.asarray(inputs["x"], np.float32).reshape(B, C)
    wmat, cvec = _prep_consts(
        inputs["w0"], inputs["kv_w"], inputs["kv_b"], inputs["w_ih"],
        inputs["w_hh"], inputs["b_ih"], inputs["b_hh"], inputs["proj_w"],
        inputs["proj_b"])

    xT = np.ascontiguousarray(x.T.astype(np.float16))  # (C, B)
    in_maps = []
    for c in range(NCORES):
        in_maps.append({
            "xT": np.ascontiguousarray(xT[:, c * BC:(c + 1) * BC]),
            "wmat": wmat,
            "cvec": cvec,
        })

    nc = _get_nc()
    res = run_bass_kernel_spmd(
        nc, in_maps, core_ids=list(range(NCORES)), trace=trace)
    outT = np.concatenate([res.results[c]["outT"] for c in range(NCORES)],
                          axis=1)  # (C, B)
    out = np.ascontiguousarray(outT.T).astype(np.float32)  # (B, C)
    return out, res


def kernel(**inputs):
    out, _ = _run(inputs, trace=False)
    return out


# revision 7
# speedup vs baseline: 1.6842x; 1.6842x over previous
"""Trainium2 Bass kernel for nn_Disease_Guide_ROI (dense_transformer).

Math notes (verified vs reference numerically):
  - softmax over a length-1 axis is exactly 1.0 => attention collapses to
    x1 = v * weight; q/k/cls_out/cls_w/cls_b are dead.
  - only the v half of the kv projection is needed.
  - the GRU update after iteration 3 is dead (weight3 unused).
  - iteration-1 gates are affine in x (hidden = w0 const): host-compose
    W_g1 = w_ih_g @ diag(w0) @ kv_v so they come straight from x.
  - with zc = 1-z (sigmoid at scale=-1):  w1 = w0 + zc1*(n1-w0),
    w2 = w1 + zc2*(n2-w1).

Precision: fp16 working tiles (~8e-4 end-to-end rel err, validated in a host
simulation), fp32 PSUM accumulation, fp32 final output.
PE observation: this environment pins the PE at 1.2 GHz and every dtype
streams 1 col/cycle, so performance = matmul count; fp16 halves DVE cost
(2x/4x modes) and input DMA bytes.
Layout: channel-major [90, N]; host pre-transposes x to (90, B) and
post-transposes the (90, B) output so every DMA moves contiguous rows.
Sharding: pure data parallel, B/8 = 16384 samples per core.
"""

import sys

if "/opt/trn_rl_repo" not in sys.path:
    sys.path.insert(0, "/opt/trn_rl_repo")

import numpy as np
from contextlib import ExitStack

B = 131072
C = 90
NCORES = 8
BC = B // NCORES  # 16384
CHUNK = 1024
NCHUNK = BC // CHUNK  # 16
MMN = 512  # matmul moving free dim (one fp32 PSUM bank)

# column indices into the per-partition fp32 constant tensor
(CV_BV, CV_W0, CV_HN1, CV_BR1, CV_NBZ1, CV_BN1, CV_BR2, CV_NBZ2, CV_BIHN,
 CV_BHHN, CV_BP) = range(11)
NCV = 11
NW = 11  # [90,90] lhsT matrices stacked in wmat

_BUILD_CACHE = {}


def _build_nc():
    import concourse.bacc as bacc
    import concourse.tile as tile
    import concourse.mybir as mybir

    f32 = mybir.dt.float32
    f16 = mybir.dt.float16
    Alu = mybir.AluOpType
    Act = mybir.ActivationFunctionType

    nc = bacc.Bacc(None, target_bir_lowering=False)
    with ExitStack() as ctx:
        tc = ctx.enter_context(tile.TileContext(nc))
        xT = nc.dram_tensor("xT", [C, BC], f16, kind="ExternalInput")
        wmat = nc.dram_tensor("wmat", [C, NW * C], f16, kind="ExternalInput")
        cvec = nc.dram_tensor("cvec", [C, NCV], f32, kind="ExternalInput")
        outT = nc.dram_tensor("outT", [C, BC], f32, kind="ExternalOutput")

        const = ctx.enter_context(tc.tile_pool(name="const", bufs=1))
        io = ctx.enter_context(tc.tile_pool(name="io", bufs=3))
        work = ctx.enter_context(tc.tile_pool(name="work", bufs=4))
        ps = ctx.enter_context(tc.tile_pool(name="ps", bufs=3, space="PSUM"))

        Wm = const.tile([C, NW * C], f16)
        nc.sync.dma_start(out=Wm, in_=wmat[:, :])
        cv = const.tile([C, NCV], f32)
        nc.sync.dma_start(out=cv, in_=cvec[:, :])

        (kvT, W1rT, W1zT, W1nT, wihT_r, wihT_z, wihT_n,
         whhT_r, whhT_z, whhT_n, projT) = (
            Wm[:, i * C:(i + 1) * C] for i in range(NW))

        def col(i):
            return cv[:, i:i + 1]

        def mm2(out_t, lhsT, rhs, start=True, stop=True):
            for h in range(CHUNK // MMN):
                nc.tensor.matmul(out_t[:, h * MMN:(h + 1) * MMN], lhsT,
                                 rhs[:, h * MMN:(h + 1) * MMN],
                                 start=start, stop=stop)

        for ch in range(NCHUNK):
            sl = slice(ch * CHUNK, (ch + 1) * CHUNK)

            x_h = io.tile([C, CHUNK], f16, tag="x")
            nc.sync.dma_start(out=x_h, in_=xT[:, sl])

            # ---- v projection ----
            pv = ps.tile([C, CHUNK], f32, tag="g")
            mm2(pv, kvT, x_h)
            v = work.tile([C, CHUNK], f16, tag="v")
            nc.scalar.activation(v, pv, Act.Identity, bias=col(CV_BV))

            # ---- iteration 1 (gates straight from x; hidden = w0) ----
            pr1 = ps.tile([C, CHUNK], f32, tag="g")
            mm2(pr1, W1rT, x_h)
            r1 = work.tile([C, CHUNK], f16, tag="g1")
            nc.scalar.activation(r1, pr1, Act.Sigmoid, bias=col(CV_BR1))

            pz1 = ps.tile([C, CHUNK], f32, tag="g")
            mm2(pz1, W1zT, x_h)
            zc1 = work.tile([C, CHUNK], f16, tag="g1")
            nc.scalar.activation(zc1, pz1, Act.Sigmoid, bias=col(CV_NBZ1),
                                 scale=-1.0)

            pi1 = ps.tile([C, CHUNK], f32, tag="g")
            mm2(pi1, W1nT, x_h)
            t2 = work.tile([C, CHUNK], f16, tag="g1")
            # t2 = (r1 * hn1c) + i_n1
            nc.vector.scalar_tensor_tensor(
                t2, r1, col(CV_HN1), pi1, Alu.mult, Alu.add)
            n1 = work.tile([C, CHUNK], f16, tag="g1")
            nc.scalar.activation(n1, t2, Act.Tanh, bias=col(CV_BN1))

            # m' = zc1*(n1 - w0);  w1 = w0 + m'
            mp = work.tile([C, CHUNK], f16, tag="g1")
            nc.vector.scalar_tensor_tensor(
                mp, n1, col(CV_W0), zc1, Alu.subtract, Alu.mult)
            w1 = work.tile([C, CHUNK], f16, tag="w")
            nc.vector.tensor_scalar(w1, mp, col(CV_W0), None, Alu.add)
            x1b = work.tile([C, CHUNK], f16, tag="x1")
            nc.vector.tensor_tensor(x1b, v, w1, Alu.mult)

            # ---- iteration 2 ----
            pr2 = ps.tile([C, CHUNK], f32, tag="g")
            mm2(pr2, wihT_r, x1b, start=True, stop=False)
            mm2(pr2, whhT_r, w1, start=False, stop=True)
            r2 = work.tile([C, CHUNK], f16, tag="g2")
            nc.scalar.activation(r2, pr2, Act.Sigmoid, bias=col(CV_BR2))

            pz2 = ps.tile([C, CHUNK], f32, tag="g")
            mm2(pz2, wihT_z, x1b, start=True, stop=False)
            mm2(pz2, whhT_z, w1, start=False, stop=True)
            zc2 = work.tile([C, CHUNK], f16, tag="g2")
            nc.scalar.activation(zc2, pz2, Act.Sigmoid, bias=col(CV_NBZ2),
                                 scale=-1.0)

            pi2 = ps.tile([C, CHUNK], f32, tag="g")
            mm2(pi2, wihT_n, x1b)
            ph2 = ps.tile([C, CHUNK], f32, tag="g")
            mm2(ph2, whhT_n, w1)

            t = work.tile([C, CHUNK], f16, tag="g2")
            nc.vector.scalar_tensor_tensor(
                t, ph2, col(CV_BHHN), r2, Alu.add, Alu.mult)
            t2b = work.tile([C, CHUNK], f16, tag="g2")
            nc.vector.tensor_tensor(t2b, t, pi2, Alu.add)
            n2 = work.tile([C, CHUNK], f16, tag="g2")
            nc.scalar.activation(n2, t2b, Act.Tanh, bias=col(CV_BIHN))

            # w2 = w1 + zc2*(n2 - w1);  x1c = v*w2
            u2 = work.tile([C, CHUNK], f16, tag="g2")
            nc.gpsimd.tensor_tensor(u2, n2, w1, Alu.subtract)
            m2 = work.tile([C, CHUNK], f16, tag="g2")
            nc.gpsimd.tensor_tensor(m2, zc2, u2, Alu.mult)
            w2 = work.tile([C, CHUNK], f16, tag="w")
            nc.vector.tensor_tensor(w2, w1, m2, Alu.add)
            x1c = work.tile([C, CHUNK], f16, tag="x1")
            nc.vector.tensor_tensor(x1c, v, w2, Alu.mult)

            # ---- output projection ----
            po = ps.tile([C, CHUNK], f32, tag="po", bufs=1)
            mm2(po, projT, x1c)
            o = io.tile([C, CHUNK], f32, tag="o")
            nc.vector.tensor_scalar(o, po, col(CV_BP), None, Alu.add)
            nc.sync.dma_start(out=outT[:, sl], in_=o)

    nc.compile()
    return nc


def _get_nc():
    if "nc" not in _BUILD_CACHE:
        _BUILD_CACHE["nc"] = _build_nc()
    return _BUILD_CACHE["nc"]


def _prep_consts(w0, kv_w, kv_b, w_ih, w_hh, b_ih, b_hh, proj_w, proj_b):
    f8 = np.float64
    w0v = np.asarray(w0, f8).reshape(C)
    kv_w = np.asarray(kv_w, f8)
    kv_b = np.asarray(kv_b, f8)
    w_ih = np.asarray(w_ih, f8)
    w_hh = np.asarray(w_hh, f8)
    b_ih = np.asarray(b_ih, f8)
    b_hh = np.asarray(b_hh, f8)
    proj_w = np.asarray(proj_w, f8)
    proj_b = np.asarray(proj_b, f8)

    kv_v = kv_w[C:2 * C]
    b_v = kv_b[C:2 * C]
    gh1 = w0v @ w_hh.T + b_hh  # iter-1 hidden gate contribution (const)

    wg = {}
    for i, g in enumerate(("r", "z", "n")):
        wg[g] = (w_ih[i * C:(i + 1) * C] * w0v[None, :]) @ kv_v

    mats = [
        kv_v.T,
        wg["r"].T, wg["z"].T, wg["n"].T,
        w_ih[0:C].T, w_ih[C:2 * C].T, w_ih[2 * C:3 * C].T,
        w_hh[0:C].T, w_hh[C:2 * C].T, w_hh[2 * C:3 * C].T,
        proj_w.T,
    ]
    wmat = np.ascontiguousarray(
        np.concatenate(mats, axis=1).astype(np.float16))

    bgate1 = {g: w_ih[i * C:(i + 1) * C] @ (w0v * b_v) + b_ih[i * C:(i + 1) * C]
              for i, g in enumerate(("r", "z", "n"))}
    cvec = np.zeros((C, NCV), np.float32)
    cvec[:, CV_BV] = b_v
    cvec[:, CV_W0] = w0v
    cvec[:, CV_HN1] = gh1[2 * C:3 * C]
    cvec[:, CV_BR1] = bgate1["r"] + gh1[0:C]
    cvec[:, CV_NBZ1] = -(bgate1["z"] + gh1[C:2 * C])
    cvec[:, CV_BN1] = bgate1["n"]
    cvec[:, CV_BR2] = b_ih[0:C] + b_hh[0:C]
    cvec[:, CV_NBZ2] = -(b_ih[C:2 * C] + b_hh[C:2 * C])
    cvec[:, CV_BIHN] = b_ih[2 * C:3 * C]
    cvec[:, CV_BHHN] = b_hh[2 * C:3 * C]
    cvec[:, CV_BP] = proj_b
    return wmat, cvec


def _run(inputs, trace=False):
    from concourse.bass_utils import run_bass_kernel_spmd

    x = np.asarray(inputs["x"], np.float32).reshape(B, C)
    wmat, cvec = _prep_consts(
        inputs["w0"], inputs["kv_w"], inputs["kv_b"], inputs["w_ih"],
        inputs["w_hh"], inputs["b_ih"], inputs["b_hh"], inputs["proj_w"],
        inputs["proj_b"])

    xT = np.ascontiguousarray(x.T.astype(np.float16))  # (C, B)
    in_maps = []
    for c in range(NCORES):
        in_maps.append({
            "xT": np.ascontiguousarray(xT[:, c * BC:(c + 1) * BC]),
            "wmat": wmat,
            "cvec": cvec,
        })

    nc = _get_nc()
    res = run_bass_kernel_spmd(
        nc, in_maps, core_ids=list(range(NCORES)), trace=trace)
    outT = np.concatenate([res.results[c]["outT"] for c in range(NCORES)],
                          axis=1)  # (C, B)
    out = np.ascontiguousarray(outT.T).astype(np.float32)  # (B, C)
    return out, res


def kernel(**inputs):
    out, _ = _run(inputs, trace=False)
    return out


# revision 8
# speedup vs baseline: 2.3540x; 1.3977x over previous
"""Trainium2 Bass kernel for nn_Disease_Guide_ROI (dense_transformer).

Math notes (verified vs reference numerically):
  - softmax over a length-1 axis is exactly 1.0 => attention collapses to
    x1 = v * weight; q/k/cls_out/cls_w/cls_b are dead.
  - only the v half of the kv projection is needed.
  - the GRU update after iteration 3 is dead (weight3 unused).
  - iteration-1 gates are affine in x (hidden = w0 const): host-compose
    W_g1 = w_ih_g @ diag(w0) @ kv_v so they come straight from x.
  - with zc = 1-z (sigmoid at scale=-1):  w1 = w0 + zc1*(n1-w0),
    w2 = w1 + zc2*(n2-w1).

Precision: fp16 working tiles (~8e-4 end-to-end rel err, host-validated),
fp32 PSUM accumulation, fp32 final output.
The PE in this environment is pinned at 1.2 GHz and streams 1 col/cycle for
all dtypes, so PE time == matmul count; fp16 buys 2x/4x DVE modes + half DMA.
Each engine executes its queue in order, so the chunk loop is emitted with a
2-stage software-pipeline skew: A(c) iter1, B(c-1) iter2, C(c-2) projection.
Layout: channel-major [90, N]; host pre-transposes x to (90, B) and
post-transposes the (90, B) output so every DMA moves contiguous rows.
Sharding: pure data parallel, B/8 = 16384 samples per core.
"""

import sys

if "/opt/trn_rl_repo" not in sys.path:
    sys.path.insert(0, "/opt/trn_rl_repo")

import numpy as np
from contextlib import ExitStack

B = 131072
C = 90
NCORES = 8
BC = B // NCORES  # 16384
CHUNK = 512
NCHUNK = BC // CHUNK  # 32

(CV_BV, CV_W0, CV_HN1, CV_BR1, CV_NBZ1, CV_BN1, CV_BR2, CV_NBZ2, CV_BIHN,
 CV_BHHN, CV_BP) = range(11)
NCV = 11
NW = 11

_BUILD_CACHE = {}


def _build_nc():
    import concourse.bacc as bacc
    import concourse.tile as tile
    import concourse.mybir as mybir

    f32 = mybir.dt.float32
    f16 = mybir.dt.float16
    Alu = mybir.AluOpType
    Act = mybir.ActivationFunctionType

    nc = bacc.Bacc(None, target_bir_lowering=False)
    with ExitStack() as ctx:
        tc = ctx.enter_context(tile.TileContext(nc))
        xT = nc.dram_tensor("xT", [C, BC], f16, kind="ExternalInput")
        wmat = nc.dram_tensor("wmat", [C, NW * C], f16, kind="ExternalInput")
        cvec = nc.dram_tensor("cvec", [C, NCV], f32, kind="ExternalInput")
        outT = nc.dram_tensor("outT", [C, BC], f32, kind="ExternalOutput")

        const = ctx.enter_context(tc.tile_pool(name="const", bufs=1))
        io = ctx.enter_context(tc.tile_pool(name="io", bufs=4))
        work = ctx.enter_context(tc.tile_pool(name="work", bufs=4))
        ps = ctx.enter_context(tc.tile_pool(name="ps", bufs=6, space="PSUM"))

        Wm = const.tile([C, NW * C], f16)
        nc.sync.dma_start(out=Wm, in_=wmat[:, :])
        cv = const.tile([C, NCV], f32)
        nc.sync.dma_start(out=cv, in_=cvec[:, :])

        (kvT, W1rT, W1zT, W1nT, wihT_r, wihT_z, wihT_n,
         whhT_r, whhT_z, whhT_n, projT) = (
            Wm[:, i * C:(i + 1) * C] for i in range(NW))

        def col(i):
            return cv[:, i:i + 1]

        # cross-stage state per in-flight chunk
        state = {}

        def stage_a(c):
            sl = slice(c * CHUNK, (c + 1) * CHUNK)
            x_h = io.tile([C, CHUNK], f16, tag="x", name="x_h")
            nc.sync.dma_start(out=x_h, in_=xT[:, sl])

            pv = ps.tile([C, CHUNK], f32, tag="g", name="pv")
            nc.tensor.matmul(pv, kvT, x_h, start=True, stop=True)
            pr1 = ps.tile([C, CHUNK], f32, tag="g", name="pr1")
            nc.tensor.matmul(pr1, W1rT, x_h, start=True, stop=True)
            pz1 = ps.tile([C, CHUNK], f32, tag="g", name="pz1")
            nc.tensor.matmul(pz1, W1zT, x_h, start=True, stop=True)
            pi1 = ps.tile([C, CHUNK], f32, tag="g", name="pi1")
            nc.tensor.matmul(pi1, W1nT, x_h, start=True, stop=True)

            v = work.tile([C, CHUNK], f16, tag="v", bufs=5, name="v")
            nc.scalar.activation(v, pv, Act.Identity, bias=col(CV_BV))
            r1 = work.tile([C, CHUNK], f16, tag="g1", bufs=6, name="r1")
            nc.scalar.activation(r1, pr1, Act.Sigmoid, bias=col(CV_BR1))
            zc1 = work.tile([C, CHUNK], f16, tag="g1", bufs=6, name="zc1")
            nc.scalar.activation(zc1, pz1, Act.Sigmoid, bias=col(CV_NBZ1),
                                 scale=-1.0)
            t2 = work.tile([C, CHUNK], f16, tag="g1", bufs=6, name="t2")
            nc.vector.scalar_tensor_tensor(
                t2, r1, col(CV_HN1), pi1, Alu.mult, Alu.add)
            n1 = work.tile([C, CHUNK], f16, tag="g1", bufs=6, name="n1")
            nc.scalar.activation(n1, t2, Act.Tanh, bias=col(CV_BN1))

            mp = work.tile([C, CHUNK], f16, tag="g1", bufs=6, name="mp")
            nc.vector.scalar_tensor_tensor(
                mp, n1, col(CV_W0), zc1, Alu.subtract, Alu.mult)
            w1 = work.tile([C, CHUNK], f16, tag="w", bufs=5, name="w1")
            nc.vector.tensor_scalar(w1, mp, col(CV_W0), None, Alu.add)
            x1b = work.tile([C, CHUNK], f16, tag="x1", bufs=5, name="x1b")
            nc.vector.tensor_tensor(x1b, v, w1, Alu.mult)
            state[c] = {"v": v, "w1": w1, "x1b": x1b}

        def stage_b(c):
            st = state[c]
            v, w1, x1b = st["v"], st["w1"], st["x1b"]

            pr2 = ps.tile([C, CHUNK], f32, tag="g", name="pr2")
            nc.tensor.matmul(pr2, wihT_r, x1b, start=True, stop=False)
            nc.tensor.matmul(pr2, whhT_r, w1, start=False, stop=True)
            pz2 = ps.tile([C, CHUNK], f32, tag="g", name="pz2")
            nc.tensor.matmul(pz2, wihT_z, x1b, start=True, stop=False)
            nc.tensor.matmul(pz2, whhT_z, w1, start=False, stop=True)
            pi2 = ps.tile([C, CHUNK], f32, tag="g", name="pi2")
            nc.tensor.matmul(pi2, wihT_n, x1b, start=True, stop=True)
            ph2 = ps.tile([C, CHUNK], f32, tag="g", name="ph2")
            nc.tensor.matmul(ph2, whhT_n, w1, start=True, stop=True)

            r2 = work.tile([C, CHUNK], f16, tag="g2", bufs=6, name="r2")
            nc.scalar.activation(r2, pr2, Act.Sigmoid, bias=col(CV_BR2))
            zc2 = work.tile([C, CHUNK], f16, tag="g2", bufs=6, name="zc2")
            nc.scalar.activation(zc2, pz2, Act.Sigmoid, bias=col(CV_NBZ2),
                                 scale=-1.0)

            t = work.tile([C, CHUNK], f16, tag="g2", bufs=6, name="t")
            nc.vector.scalar_tensor_tensor(
                t, ph2, col(CV_BHHN), r2, Alu.add, Alu.mult)
            t2b = work.tile([C, CHUNK], f16, tag="g2", bufs=6, name="t2b")
            nc.vector.tensor_tensor(t2b, t, pi2, Alu.add)
            n2 = work.tile([C, CHUNK], f16, tag="g2", bufs=6, name="n2")
            nc.scalar.activation(n2, t2b, Act.Tanh, bias=col(CV_BIHN))

            u2 = work.tile([C, CHUNK], f16, tag="g2", bufs=6, name="u2")
            nc.gpsimd.tensor_tensor(u2, n2, w1, Alu.subtract)
            m2 = work.tile([C, CHUNK], f16, tag="g2", bufs=6, name="m2")
            nc.gpsimd.tensor_tensor(m2, zc2, u2, Alu.mult)
            w2 = work.tile([C, CHUNK], f16, tag="w", bufs=5, name="w2")
            nc.vector.tensor_tensor(w2, w1, m2, Alu.add)
            x1c = work.tile([C, CHUNK], f16, tag="x1", bufs=5, name="x1c")
            nc.vector.tensor_tensor(x1c, v, w2, Alu.mult)
            st["x1c"] = x1c

        def stage_c(c):
            st = state.pop(c)
            sl = slice(c * CHUNK, (c + 1) * CHUNK)
            po = ps.tile([C, CHUNK], f32, tag="po", bufs=2, name="po")
            nc.tensor.matmul(po, projT, st["x1c"], start=True, stop=True)
            o = io.tile([C, CHUNK], f32, tag="o", name="o")
            nc.vector.tensor_scalar(o, po, col(CV_BP), None, Alu.add)
            nc.sync.dma_start(out=outT[:, sl], in_=o)

        for c in range(NCHUNK + 2):
            if c < NCHUNK:
                stage_a(c)
            if 1 <= c <= NCHUNK:
                stage_b(c - 1)
            if 2 <= c:
                stage_c(c - 2)

    nc.compile()
    return nc


def _get_nc():
    if "nc" not in _BUILD_CACHE:
        _BUILD_CACHE["nc"] = _build_nc()
    return _BUILD_CACHE["nc"]


def _prep_consts(w0, kv_w, kv_b, w_ih, w_hh, b_ih, b_hh, proj_w, proj_b):
    f8 = np.float64
    w0v = np.asarray(w0, f8).reshape(C)
    kv_w = np.asarray(kv_w, f8)
    kv_b = np.asarray(kv_b, f8)
    w_ih = np.asarray(w_ih, f8)
    w_hh = np.asarray(w_hh, f8)
    b_ih = np.asarray(b_ih, f8)
    b_hh = np.asarray(b_hh, f8)
    proj_w = np.asarray(proj_w, f8)
    proj_b = np.asarray(proj_b, f8)

    kv_v = kv_w[C:2 * C]
    b_v = kv_b[C:2 * C]
    gh1 = w0v @ w_hh.T + b_hh  # iter-1 hidden gate contribution (const)

    wg = {}
    for i, g in enumerate(("r", "z", "n")):
        wg[g] = (w_ih[i * C:(i + 1) * C] * w0v[None, :]) @ kv_v

    mats = [
        kv_v.T,
        wg["r"].T, wg["z"].T, wg["n"].T,
        w_ih[0:C].T, w_ih[C:2 * C].T, w_ih[2 * C:3 * C].T,
        w_hh[0:C].T, w_hh[C:2 * C].T, w_hh[2 * C:3 * C].T,
        proj_w.T,
    ]
    wmat = np.ascontiguousarray(
        np.concatenate(mats, axis=1).astype(np.float16))

    bgate1 = {g: w_ih[i * C:(i + 1) * C] @ (w0v * b_v) + b_ih[i * C:(i + 1) * C]
              for i, g in enumerate(("r", "z", "n"))}
    cvec = np.zeros((C, NCV), np.float32)
    cvec[:, CV_BV] = b_v
    cvec[:, CV_W0] = w0v
    cvec[:, CV_HN1] = gh1[2 * C:3 * C]
    cvec[:, CV_BR1] = bgate1["r"] + gh1[0:C]
    cvec[:, CV_NBZ1] = -(bgate1["z"] + gh1[C:2 * C])
    cvec[:, CV_BN1] = bgate1["n"]
    cvec[:, CV_BR2] = b_ih[0:C] + b_hh[0:C]
    cvec[:, CV_NBZ2] = -(b_ih[C:2 * C] + b_hh[C:2 * C])
    cvec[:, CV_BIHN] = b_ih[2 * C:3 * C]
    cvec[:, CV_BHHN] = b_hh[2 * C:3 * C]
    cvec[:, CV_BP] = proj_b
    return wmat, cvec


def _run(inputs, trace=False):
    from concourse.bass_utils import run_bass_kernel_spmd

    x = np.asarray(inputs["x"], np.float32).reshape(B, C)
    wmat, cvec = _prep_consts(
        inputs["w0"], inputs["kv_w"], inputs["kv_b"], inputs["w_ih"],
        inputs["w_hh"], inputs["b_ih"], inputs["b_hh"], inputs["proj_w"],
        inputs["proj_b"])

    xT = np.ascontiguousarray(x.T.astype(np.float16))  # (C, B)
    in_maps = []
    for c in range(NCORES):
        in_maps.append({
            "xT": np.ascontiguousarray(xT[:, c * BC:(c + 1) * BC]),
            "wmat": wmat,
            "cvec": cvec,
        })

    nc = _get_nc()
    res = run_bass_kernel_spmd(
        nc, in_maps, core_ids=list(range(NCORES)), trace=trace)
    outT = np.concatenate([res.results[c]["outT"] for c in range(NCORES)],
                          axis=1)  # (C, B)
    out = np.ascontiguousarray(outT.T).astype(np.float32)  # (B, C)
    return out, res


def kernel(**inputs):
    out, _ = _run(inputs, trace=False)
    return out
